# revision 1
# baseline (speedup 1.0000x reference)
"""GRU free-run greedy decoder on 8 Trainium2 NeuronCores (data parallel).

Problem: 2-layer GRU (H=512) + fc(V=256) greedy decode, T=64 steps,
B=1024 batch, latent LAT=256 concatenated with previous one-hot as input.

Sharding: pure data parallel. Each of the 8 cores handles 128 batch rows
(= exactly the 128 SBUF partitions). GRU + fc weights are replicated.
The whole recurrence runs on-chip: weights, hidden state, and per-step
one-hots all live in SBUF; only the final [128, T, V] one-hot stream is
DMA'd out.

Matmul mapping ("mapping 1"): out[batch, outdim] = lhsT.T @ rhs with
  lhsT (stationary) = activation^T chunk [K=128, 128 batch]
  rhs  (moving)     = weight^T chunk     [K=128, <=512 outdim]
so the PE streams the (large) weight operand and the per-step activation
transposes are small PE transpose ops. h-dependent matmuls are fp32 (the
argmax trajectory needs fp32-faithful logits; bf16/tf32/f32r flip tokens
— measured). One-hot embedding and bias adds run as EXACT 3-way bf16
decompositions at full PE rate, seeded into PSUM by prefetchable
identity-matmuls.

Host-side prep (layout + exact splits): weight transposes / reshapes,
bias combination + replication across partitions.
"""

import sys
import numpy as np

sys.path.insert(0, "/opt/trn_rl_repo")

P = 128          # partitions == per-core batch
H = 512          # hidden
V = 256          # vocab
LAT = 256        # latent dim
G3 = 3 * H       # 1536 gate width
T_FULL = 64
N_CORES = 8

_CACHE = {}


def build_program(T=T_FULL, use_f32r=False):
    """Build + compile the Bass program. Returns the compiled Bacc object."""
    import concourse.bass as bass
    import concourse.tile as tile
    from concourse import bacc, mybir
    from concourse.masks import make_identity

    f32 = mybir.dt.float32
    bf16 = mybir.dt.bfloat16
    f16 = mybir.dt.float16
    mm = mybir.dt.float32r if use_f32r else f32
    AF = mybir.ActivationFunctionType
    OP = mybir.AluOpType
    ts = bass.ts

    nc = bacc.Bacc(
        "TRN2", target_bir_lowering=False, debug=False,
        enable_asserts=False, num_devices=N_CORES,
    )

    # ---- DRAM I/O ----
    # one-hot/identity/bias matmuls run as EXACT 3-way bf16 decompositions
    # (8+8+8 mantissa bits cover fp32's 24; measured: even 2^-18 rounding
    # here flips argmax rows). h-dependent gate/fc matmuls stay fp32.
    lat_d = nc.dram_tensor("lat", [P, LAT], f32, kind="ExternalInput").ap()
    wlatT_d = nc.dram_tensor("wlatT", [2, P, G3], mm, kind="ExternalInput").ap()
    wembTh_d = nc.dram_tensor("wembTh", [2, P, G3], f16, kind="ExternalInput").ap()
    wembTl_d = nc.dram_tensor("wembTl", [2, P, G3], f16, kind="ExternalInput").ap()
    whh0T_d = nc.dram_tensor("whh0T", [4, P, G3], mm, kind="ExternalInput").ap()
    wih1T_d = nc.dram_tensor("wih1T", [4, P, G3], mm, kind="ExternalInput").ap()
    whh1T_d = nc.dram_tensor("whh1T", [4, P, G3], mm, kind="ExternalInput").ap()
    wfcT_d = nc.dram_tensor("wfcT", [4, P, V], mm, kind="ExternalInput").ap()
    blc_d = nc.dram_tensor("blc", [P, G3], f32, kind="ExternalInput").ap()
    b0hn_d = nc.dram_tensor("b0hn", [P, H], f32, kind="ExternalInput").ap()
    b1rzs_d = nc.dram_tensor("b1rzs", [3, 2 * H], bf16, kind="ExternalInput").ap()
    b1in_d = nc.dram_tensor("b1in", [P, H], f32, kind="ExternalInput").ap()
    b1hn_d = nc.dram_tensor("b1hn", [P, H], f32, kind="ExternalInput").ap()
    bfcs_d = nc.dram_tensor("bfcs", [3, V], bf16, kind="ExternalInput").ap()
    out_d = nc.dram_tensor("out", [P, T, V], f32, kind="ExternalOutput").ap()

    from contextlib import ExitStack
    with tile.TileContext(nc) as tc, ExitStack() as ctx:
        wt = ctx.enter_context(tc.tile_pool(name="wt", bufs=1))
        st = ctx.enter_context(tc.tile_pool(name="st", bufs=1))
        wk = ctx.enter_context(tc.tile_pool(name="wk", bufs=2))
        # PSUM: 8 banks total. rz gates 2x[P,1024] (4 banks) double-buffered;
        # ihn (i_n|h_n) single slot; tail (transposes/fc/ohT) single slot.
        ps = ctx.enter_context(tc.tile_pool(name="ps", bufs=2, space="PSUM"))
        ps1 = ctx.enter_context(tc.tile_pool(name="ps1", bufs=1, space="PSUM"))

        # ---- persistent weights/biases in SBUF ----
        whh0T = wt.tile([P, 4, G3], mm, tag="whh0T")
        wih1T = wt.tile([P, 4, G3], mm, tag="wih1T")
        whh1T = wt.tile([P, 4, G3], mm, tag="whh1T")
        wembTh = wt.tile([P, 2, G3], f16, tag="wembTh")
        wembTl = wt.tile([P, 2, G3], f16, tag="wembTl")
        wlatT = wt.tile([P, 2, G3], mm, tag="wlatT")
        wfcT = wt.tile([P, 4, V], mm, tag="wfcT")
        # DMA order matches first-use: step 0 needs the Lc chain + wih1T
        # + wfcT (hh0/gh1/emb are skipped at t=0), step 1 the rest.
        latsb = wt.tile([P, LAT], f32, tag="latsb")
        nc.sync.dma_start(latsb[:], lat_d[:])
        for kc in range(2):
            nc.sync.dma_start(wlatT[:, kc, :], wlatT_d[kc])
        for kc in range(4):
            nc.sync.dma_start(wih1T[:, kc, :], wih1T_d[kc])
        for kc in range(4):
            nc.sync.dma_start(wfcT[:, kc, :], wfcT_d[kc])
        for kc in range(4):
            nc.sync.dma_start(whh0T[:, kc, :], whh0T_d[kc])
            nc.sync.dma_start(whh1T[:, kc, :], whh1T_d[kc])
        for kc in range(2):
            nc.sync.dma_start(wembTh[:, kc, :], wembTh_d[kc])
            nc.sync.dma_start(wembTl[:, kc, :], wembTl_d[kc])

        blc = wt.tile([P, G3], f32, tag="blc")
        b0hn = wt.tile([P, H], f32, tag="b0hn")
        b1rzs = wt.tile([3, 2 * H], bf16, tag="b1rzs")
        b1in = wt.tile([P, H], f32, tag="b1in")
        b1hn = wt.tile([P, H], f32, tag="b1hn")
        bfcs = wt.tile([3, V], bf16, tag="bfcs")
        nc.sync.dma_start(blc[:], blc_d[:])
        nc.sync.dma_start(b0hn[:], b0hn_d[:])
        nc.sync.dma_start(b1rzs[:], b1rzs_d[:])
        nc.sync.dma_start(b1in[:], b1in_d[:])
        nc.sync.dma_start(b1hn[:], b1hn_d[:])
        nc.sync.dma_start(bfcs[:], bfcs_d[:])

        ones3 = wt.tile([3, P], bf16, tag="ones3")
        nc.gpsimd.memset(ones3[:], 1.0)
        zer = wt.tile([P, H], bf16, tag="zer")
        nc.gpsimd.memset(zer[:], 0.0)
        ident = wt.tile([P, P], f32, tag="ident")
        make_identity(nc, ident[:])
        identb = wt.tile([P, P], bf16, tag="identb")
        make_identity(nc, identb[:])
        identf = wt.tile([P, P], f16, tag="identf")
        make_identity(nc, identf[:])
        ident2 = wt.tile([P, P], f16, tag="ident2")  # 2^-12 diagonal
        nc.gpsimd.memset(ident2[:], 0.0)
        nc.gpsimd.affine_select(
            out=ident2[:], in_=ident2[:],
            compare_op=mybir.AluOpType.not_equal, fill=2.0 ** -12,
            base=0, pattern=[[-1, P]], channel_multiplier=1)

        # ---- persistent state ----
        h0 = st.tile([P, H], f32, tag="h0")
        h1 = st.tile([P, H], f32, tag="h1")
        h0T = st.tile([P, 4, P], mm, tag="h0T")
        h1T = st.tile([P, 4, P], mm, tag="h1T")
        ohT = st.tile([P, 2, P], f16, tag="ohT")
        ohT2 = st.tile([P, 2, P], f16, tag="ohT2")  # one-hot * 2^-12
        Lc = st.tile([P, G3], f32, tag="Lc")
        Lch = st.tile([P, 2 * H], f16, tag="Lch")  # rz part, fp16 hi
        Lcl = st.tile([P, 2 * H], f16, tag="Lcl")  # rz part, fp16 lo*2^12
        for tl in (h0, h1):
            nc.gpsimd.memset(tl[:], 0.0)
        nc.gpsimd.memset(h0T[:, :, :], 0.0)
        nc.gpsimd.memset(h1T[:, :, :], 0.0)
        nc.gpsimd.memset(ohT[:, :, :], 0.0)
        nc.gpsimd.memset(ohT2[:, :, :], 0.0)

        # ---- setup: Lc = latent @ WlatT + (b_ih0 + b_hh0 (rz-only)) ----
        s1 = ps.tile([P, 1024], f32, tag="rz")
        latT = wt.tile([P, 2, P], mm, tag="latT")
        for kc in range(2):
            nc.tensor.transpose(s1[:, ts(kc, P)], latsb[:, ts(kc, P)], ident[:])
        nc.scalar.copy(latT[:, :, :].rearrange("p a b -> p (a b)"), s1[:, 0:256])

        s2 = ps.tile([P, 1024], f32, tag="rz")
        s3 = ps1.tile([P, 1024], f32, tag="ihn")
        for kc in range(2):
            for j in range(2):
                nc.tensor.matmul(s2[:, ts(j, 512)], latT[:, kc, :],
                                 wlatT[:, kc, ts(j, 512)],
                                 start=(kc == 0), stop=(kc == 1))
            nc.tensor.matmul(s3[:, 0:512], latT[:, kc, :],
                             wlatT[:, kc, 1024:1536],
                             start=(kc == 0), stop=(kc == 1))
        nc.vector.tensor_add(Lc[:, 0:1024], s2[:, 0:1024], blc[:, 0:1024])
        nc.vector.tensor_add(Lc[:, 1024:1536], s3[:, 0:512], blc[:, 1024:1536])
        # split the rz part into fp16 hi + (lo * 2^12): 22+ bits, the
        # ~2e-10 residual is orders below the min argmax gap (measured)
        nc.vector.tensor_copy(Lch[:], Lc[:, 0:1024])
        Lchf = wt.tile([P, 2 * H], f32, tag="Lchf")
        nc.vector.tensor_copy(Lchf[:], Lch[:])
        r1 = wt.tile([P, 2 * H], f32, tag="r1")
        nc.vector.tensor_sub(r1[:], Lc[:, 0:1024], Lchf[:])
        nc.scalar.mul(Lcl[:], r1[:], 2.0 ** 12)

        # ---- helper: emit one accumulation group ----
        def mm_group(dest, contribs):
            n = len(contribs)
            for i, (lhsT, rhs) in enumerate(contribs):
                nc.tensor.matmul(dest, lhsT, rhs,
                                 start=(i == 0), stop=(i == n - 1))

        def gru_gates(grz, gihn, lc_in, bhn, h, tag):
            """gates + state update for one layer; h updated in place.
            grz psum [P,1024] already holds bias + gi_rz + gh_rz (bias was
            seeded by an identity-matmul), so sigmoid reads PSUM directly.
            gihn psum: [gi_n | gh_n]; lc_in/bhn are fp32 sbuf adds."""
            rr = wk.tile([P, H], f32, tag="rr", name=f"rr{tag}")
            nc.scalar.activation(rr[:], grz[:, 0:512], AF.Sigmoid)
            # off-critical-path adds overlap the sigmoid
            hn = wk.tile([P, H], f32, tag="hn", name=f"hn{tag}")
            nc.vector.tensor_add(hn[:], gihn[:, 512:1024], bhn)
            inn = wk.tile([P, H], f32, tag="inn", name=f"inn{tag}")
            nc.vector.tensor_add(inn[:], gihn[:, 0:512], lc_in)
            zz = wk.tile([P, H], f32, tag="zz", name=f"zz{tag}")
            nc.scalar.activation(zz[:], grz[:, 512:1024], AF.Sigmoid)
            rhn = wk.tile([P, H], f32, tag="rhn", name=f"rhn{tag}")
            nc.vector.tensor_mul(rhn[:], rr[:], hn[:])
            npre = wk.tile([P, H], f32, tag="npre", name=f"npre{tag}")
            nc.vector.tensor_add(npre[:], inn[:], rhn[:])
            nn = wk.tile([P, H], f32, tag="nn", name=f"nn{tag}")
            nc.scalar.activation(nn[:], npre[:], AF.Tanh)
            # h' = n + z*(h - n)
            dd = wk.tile([P, H], f32, tag="dd", name=f"dd{tag}")
            nc.vector.tensor_sub(dd[:], h[:], nn[:])
            zd = wk.tile([P, H], f32, tag="zd", name=f"zd{tag}")
            nc.vector.tensor_mul(zd[:], zz[:], dd[:])
            # final add per 128-chunk so each transpose starts asap
            for kc in range(4):
                sl = slice(kc * P, (kc + 1) * P)
                nc.vector.tensor_add(h[:, sl], nn[:, sl], zd[:, sl])

        def bias_seed(dest, parts, stop=False):
            """Seed a psum region with a replicated bias via identity
            matmuls. `parts` is an EXACT 3-way bf16 decomposition, so this
            is bit-identical to adding the fp32 bias; it starts the
            region's accumulation group as prefetchable PE work, removing
            a DVE add from the critical path."""
            n = dest.shape[-1]
            for pi, (stat, part) in enumerate(parts):  # chunk-inner order
                for ci in range(0, n, 512):
                    w = min(512, n - ci)
                    nc.tensor.matmul(dest[:, ci:ci + w], stat[:],
                                     part[:, ci:ci + w], start=(pi == 0),
                                     stop=(stop and pi == len(parts) - 1))

        def stack_seed(dest, stk, stop=False):
            """Seed a psum region with a partition-constant bias in ONE
            K=3 matmul: ones[3,128].T @ stacked-parts[3,N]. The three bf16
            parts sum exactly to the fp32 bias (any add order)."""
            n = dest.shape[-1]
            for ci in range(0, n, 512):
                w = min(512, n - ci)
                nc.tensor.matmul(dest[:, ci:ci + w], ones3[:],
                                 stk[:, ci:ci + w], start=True, stop=stop)

        # ---- the T decode steps, software-pipelined so the PE never idles:
        # step t's hh0/gh1 matmuls are emitted before step t-1's argmax /
        # onehot tail, so the PE chews on them while DVE finishes t-1.
        # (t=0 works uniformly because state/ohT start zeroed.) ----
        def argmax_tail(t, tail, lg):
            """argmax(lg, psum) -> one-hot (first max wins) -> DMA + ohT."""
            mx = wk.tile([P, 1], f32, tag="mx", name=f"mx_{t}")
            nc.vector.reduce_max(mx[:], lg, axis=mybir.AxisListType.X)
            ohraw = wk.tile([P, V], f32, tag="ohraw", name=f"ohraw_{t}")
            nc.vector.tensor_scalar(ohraw[:], lg, mx[:, 0:1], None,
                                    op0=OP.is_equal)
            cum = wk.tile([P, V], f32, tag="cum", name=f"cum_{t}")
            nc.vector.tensor_tensor_scan(cum[:], ohraw[:], ohraw[:], 0.0,
                                         op0=OP.add, op1=OP.bypass)
            oh = wk.tile([P, V], f32, tag="oh", name=f"oh_{t}")
            nc.vector.scalar_tensor_tensor(oh[:], cum[:], 1.0, ohraw[:],
                                           op0=OP.is_equal, op1=OP.mult)
            nc.sync.dma_start(out_d[:, t, :], oh[:])
            if tail is not None:
                for v in range(2):
                    nc.tensor.transpose(tail[:, 768 + v * P:768 + (v + 1) * P],
                                        oh[:, ts(v, P)], ident[:])
                nc.scalar.copy(ohT[:, :, :].rearrange("p a b -> p (a b)"),
                               tail[:, 768:1024])
                nc.scalar.mul(ohT2[:, :, :].rearrange("p a b -> p (a b)"),
                              tail[:, 768:1024], 2.0 ** -12)

        prev_tail = None
        prev_lg = None
        ng0rz = ng1rz = None
        for t in range(T):
            # -- prefetchable matmuls for step t (fill PE during t-1 tail);
            # rz seeds were already emitted during step t-1's l1 gates --
            if t == 0:
                g0rz = ps.tile([P, 1024], f32, tag="rz", name="g0rz_0")
                bias_seed(g0rz, ((identf, Lch), (ident2, Lcl)), stop=True)
            else:
                g0rz = ng0rz
            g0ihn = ps1.tile([P, 1024], f32, tag="ihn", name=f"g0ihn_{t}")
            # at t=0 h/onehot are zero: skip their matmuls, zero-seed the
            # psum regions they would have started instead.
            if t > 0:
                # kc-major: consecutive matmuls target different psum banks
                # (r | z | h_n), letting fill/drain overlap
                for kc in range(4):
                    for j in range(2):   # rz: accumulate onto the Lc seed
                        nc.tensor.matmul(g0rz[:, ts(j, 512)], h0T[:, kc, :],
                                         whh0T[:, kc, ts(j, 512)],
                                         start=False, stop=False)
                    nc.tensor.matmul(g0ihn[:, 512:1024], h0T[:, kc, :],
                                     whh0T[:, kc, 1024:1536],
                                     start=(kc == 0), stop=(kc == 3))
            else:
                nc.tensor.matmul(g0ihn[:, 512:1024], identb[:], zer[:],
                                 start=True, stop=True)
            if t == 0:
                g1rz = ps.tile([P, 1024], f32, tag="rz", name="g1rz_0")
                stack_seed(g1rz, b1rzs)
            else:
                g1rz = ng1rz
            if t > 0:
                for kc in range(4):   # gh1 rz, kc-major (bank alternation)
                    for j in range(2):
                        nc.tensor.matmul(g1rz[:, ts(j, 512)], h1T[:, kc, :],
                                         whh1T[:, kc, ts(j, 512)],
                                         start=False, stop=False)

            # -- step t-1 tail: argmax -> one-hot -> ohT (DVE/ACT work) --
            if t > 0:
                argmax_tail(t - 1, prev_tail, prev_lg)

            # -- emb finishes layer0 groups (needs ohT from t-1 tail);
            #    EXACT 3-way bf16; regions complete in chain-priority
            #    order r -> i_n -> z (z is only needed at the blend) --
            if t > 0:
                passes = ((ohT, wembTh), (ohT2, wembTl))
                for j in (0, None, 1):
                    if j is None:
                        mm_group(g0ihn[:, 0:512],
                                 [(oh_s[:, v, :], hl[:, v, 1024:1536])
                                  for oh_s, hl in passes for v in range(2)])
                        continue
                    for hl_i, (oh_s, hl) in enumerate(passes):
                        for vi, v in enumerate(range(2)):
                            nc.tensor.matmul(g0rz[:, ts(j, 512)],
                                             oh_s[:, v, :],
                                             hl[:, v, ts(j, 512)],
                                             start=False,
                                             stop=(hl_i == 1 and vi == 1))
            else:
                nc.tensor.matmul(g0ihn[:, 0:512], identb[:], zer[:],
                                 start=True, stop=True)

            # -- layer0 gates -> h0 (in place) --
            gru_gates(g0rz, g0ihn, Lc[:, 1024:1536], b0hn[:], h0, f"0_{t}")

            # -- gh1 h_n first: it is runnable while DVE computes the l0
            # gates, unlike the h0'^T transposes queued next (PE in-order) --
            g1ihn = ps1.tile([P, 1024], f32, tag="ihn", name=f"g1ihn_{t}")
            if t > 0:
                mm_group(g1ihn[:, 512:1024],
                         [(h1T[:, kc, :], whh1T[:, kc, 1024:1536])
                          for kc in range(4)])
            else:
                nc.tensor.matmul(g1ihn[:, 512:1024], identb[:], zer[:],
                                 start=True, stop=True)

            # -- h0'^T -> tail psum -> h0T (per-chunk so gi1 starts asap) --
            tail = ps1.tile([P, 1024], f32, tag="tail", name=f"tail_{t}")
            for kc in range(4):
                nc.tensor.transpose(tail[:, ts(kc, P)], h0[:, ts(kc, P)], ident[:])
                nc.scalar.copy(h0T[:, kc, :], tail[:, ts(kc, P)])

            # -- gi1 (= h0' @ Wih1T), regions r -> i_n -> z --
            for j in (0, None, 1):
                if j is None:
                    mm_group(g1ihn[:, 0:512],
                             [(h0T[:, kc, :], wih1T[:, kc, 1024:1536])
                              for kc in range(4)])
                    continue
                for kc in range(4):
                    nc.tensor.matmul(g1rz[:, ts(j, 512)], h0T[:, kc, :],
                                     wih1T[:, kc, ts(j, 512)],
                                     start=False, stop=(kc == 3))

            # -- layer1 gates -> h1 (in place) --
            gru_gates(g1rz, g1ihn, b1in[:], b1hn[:], h1, f"1_{t}")

            # -- step t+1 bias seeds: runnable during the l1 gate chain
            # (their rz slots free right after this step's sigmoids) --
            if t + 1 < T:
                ng0rz = ps.tile([P, 1024], f32, tag="rz", name=f"g0rz_{t+1}")
                bias_seed(ng0rz, ((identf, Lch), (ident2, Lcl)), stop=False)
                ng1rz = ps.tile([P, 1024], f32, tag="rz", name=f"g1rz_{t+1}")
                stack_seed(ng1rz, b1rzs)

            # -- h1'^T -> tail (reuse cols [0:512]) -> h1T --
            for kc in range(4):
                nc.tensor.transpose(tail[:, ts(kc, P)], h1[:, ts(kc, P)], ident[:])
                nc.scalar.copy(h1T[:, kc, :], tail[:, ts(kc, P)])

            # -- fc logits (+bias via seed) -> tail cols [512:768] --
            stack_seed(tail[:, 512:768], bfcs)
            for kc in range(4):
                nc.tensor.matmul(tail[:, 512:768], h1T[:, kc, :],
                                 wfcT[:, kc, :], start=False, stop=(kc == 3))
            prev_tail, prev_lg = tail, tail[:, 512:768]

        argmax_tail(T - 1, None, prev_lg)

    nc.compile()
    return nc


def prep_host_inputs(latent_vec, w_ih0, w_hh0, b_ih0, b_hh0,
                     w_ih_r, w_hh_r, b_ih_r, b_hh_r, w_fc, b_fc,
                     use_f32r=False):
    """Pure-layout host prep: transposes/reshapes + bias merge/replicate.
    Returns per-core in_maps."""
    import ml_dtypes
    f4 = np.float32
    bf = ml_dtypes.bfloat16

    def rep(v):  # replicate a [N] vector across the 128 partitions
        return np.ascontiguousarray(np.broadcast_to(v.astype(f4), (P, v.shape[0])))

    def split_bf16(a):  # EXACT 3-way bf16 split: a == h + m + l in fp32
        a = a.astype(f4)
        hi = a.astype(bf)
        r = a - hi.astype(f4)
        mid = r.astype(bf)
        lo = (r - mid.astype(f4)).astype(bf)
        return (np.ascontiguousarray(hi), np.ascontiguousarray(mid),
                np.ascontiguousarray(lo))

    wlatT = np.ascontiguousarray(w_ih0[:, :LAT].T.astype(f4)).reshape(2, P, G3)
    wembT = np.ascontiguousarray(w_ih0[:, LAT:].T.astype(f4)).reshape(2, P, G3)
    f16 = np.float16
    wembTh = wembT.astype(f16)
    wembTl = ((wembT - wembTh.astype(f4)) * 4096.0).astype(f16)
    wembTh, wembTl = np.ascontiguousarray(wembTh), np.ascontiguousarray(wembTl)
    whh0T = np.ascontiguousarray(w_hh0.T.astype(f4)).reshape(4, P, G3)
    wih1T = np.ascontiguousarray(w_ih_r[0].T.astype(f4)).reshape(4, P, G3)
    whh1T = np.ascontiguousarray(w_hh_r[0].T.astype(f4)).reshape(4, P, G3)
    wfcT = np.ascontiguousarray(w_fc.T.astype(f4)).reshape(4, P, V)

    blc_v = b_ih0.astype(f4).copy()
    blc_v[:1024] += b_hh0[:1024].astype(f4)
    b1rzs = np.ascontiguousarray(
        np.stack(split_bf16((b_ih_r[0] + b_hh_r[0])[:1024])))
    bfcs = np.ascontiguousarray(np.stack(split_bf16(b_fc)))
    common = dict(
        wlatT=wlatT, wembTh=wembTh, wembTl=wembTl,
        whh0T=whh0T, wih1T=wih1T, whh1T=whh1T,
        wfcT=wfcT, blc=rep(blc_v), b0hn=rep(b_hh0[1024:]),
        b1rzs=b1rzs,
        b1in=rep(b_ih_r[0][1024:]), b1hn=rep(b_hh_r[0][1024:]),
        bfcs=bfcs,
    )
    in_maps = []
    for c in range(N_CORES):
        m = dict(common)
        m["lat"] = np.ascontiguousarray(latent_vec[c * P:(c + 1) * P].astype(f4))
        in_maps.append(m)
    return in_maps


def kernel(**inputs):
    from concourse import bass_utils

    use_f32r = _CACHE.get("use_f32r", False)
    key = ("prog", T_FULL, use_f32r)
    if key not in _CACHE:
        _CACHE[key] = build_program(T_FULL, use_f32r=use_f32r)
    nc = _CACHE[key]

    in_maps = prep_host_inputs(
        np.asarray(inputs["latent_vec"]), np.asarray(inputs["w_ih0"]),
        np.asarray(inputs["w_hh0"]), np.asarray(inputs["b_ih0"]),
        np.asarray(inputs["b_hh0"]), np.asarray(inputs["w_ih_r"]),
        np.asarray(inputs["w_hh_r"]), np.asarray(inputs["b_ih_r"]),
        np.asarray(inputs["b_hh_r"]), np.asarray(inputs["w_fc"]),
        np.asarray(inputs["b_fc"]), use_f32r=use_f32r)

    res = bass_utils.run_bass_kernel_spmd(nc, in_maps, list(range(N_CORES)))
    out = np.concatenate([res.results[c]["out"] for c in range(N_CORES)], axis=0)
    return out.astype(np.float32)



# revision 7
# speedup vs baseline: 1.0675x; 1.0675x over previous
"""GRU free-run greedy decoder on 8 Trainium2 NeuronCores (data parallel).

Problem: 2-layer GRU (H=512) + fc(V=256) greedy decode, T=64 steps,
B=1024 batch, latent LAT=256 concatenated with previous one-hot as input.

Sharding: pure data parallel. Each of the 8 cores handles 128 batch rows
(= exactly the 128 SBUF partitions). GRU + fc weights are replicated.
The whole recurrence runs on-chip; only the final [128, T, V] one-hot
stream is DMA'd out (as fp16, exact for one-hots).

Matmul mapping: out[batch, outdim] = lhsT.T @ rhs with
  lhsT (stationary) = activation^T chunk [K=128, 128 batch]
  rhs  (moving)     = weight^T chunk     [K=128, <=512 outdim]

Precision: h-dependent matmuls (hh0 / ih1 / hh1 / fc) run as 3-term fp16
split products accumulated in fp32 PSUM:
    h @ W ~= a@c + (a*2^-12)@d_s + b@c,
    a=f16(h), b=f16(h-a), c=f16(W), d_s=f16((W-c)*2^12)
The scaled d_s pair keeps the W residual at full fp16 precision (W
captured to ~2^-23); b sits partly in fp16 subnormal range, which the PE
honors exactly (verified by HW probe) and is quantum-2^-24-absolute, so
h is captured to ~2^-24 too. The numpy-emulated trajectory of this
scheme matches the fp64 reference argmax for every token, while all
1/2-term variants flip hundreds of tokens. Cost: 12 fp16 chunk-streams
per 512-K matmul vs fp32's 16 cycle-equivalents, i.e. 25% less PE time
on the dominant matmuls plus cheaper transposes/copies.

One-hot embedding stays an EXACT 2-pass fp16 scheme; Lc / layer-1 / fc
biases are added on the Vector engine (frees all PE bias-seed matmuls).
"""

import sys
import numpy as np

sys.path.insert(0, "/opt/trn_rl_repo")

P = 128          # partitions == per-core batch
H = 512          # hidden
V = 256          # vocab
LAT = 256        # latent dim
G3 = 3 * H       # 1536 gate width
T_FULL = 64
N_CORES = 8

_CACHE = {}


def build_program(T=T_FULL):
    """Build + compile the Bass program. Returns the compiled Bacc object."""
    import concourse.bass as bass
    import concourse.tile as tile
    from concourse import bacc, mybir
    from concourse.masks import make_identity

    f32 = mybir.dt.float32
    f16 = mybir.dt.float16
    bf16 = mybir.dt.bfloat16
    AF = mybir.ActivationFunctionType
    OP = mybir.AluOpType
    ts = bass.ts

    nc = bacc.Bacc(
        "TRN2", target_bir_lowering=False, debug=False,
        enable_asserts=False, num_devices=N_CORES,
    )

    # ---- DRAM I/O ----
    lat_d = nc.dram_tensor("lat", [P, LAT], f32, kind="ExternalInput").ap()
    wlatT_d = nc.dram_tensor("wlatT", [2, P, G3], f32, kind="ExternalInput").ap()
    wembh_d = nc.dram_tensor("wembh", [2, P, G3], f16, kind="ExternalInput").ap()
    wembl_d = nc.dram_tensor("wembl", [2, P, G3], f16, kind="ExternalInput").ap()
    whh0c_d = nc.dram_tensor("whh0c", [4, P, G3], f16, kind="ExternalInput").ap()
    whh0d_d = nc.dram_tensor("whh0d", [4, P, G3], f16, kind="ExternalInput").ap()
    wih1c_d = nc.dram_tensor("wih1c", [4, P, G3], f16, kind="ExternalInput").ap()
    wih1d_d = nc.dram_tensor("wih1d", [4, P, G3], f16, kind="ExternalInput").ap()
    whh1c_d = nc.dram_tensor("whh1c", [4, P, G3], f16, kind="ExternalInput").ap()
    whh1d_d = nc.dram_tensor("whh1d", [4, P, G3], f16, kind="ExternalInput").ap()
    wfcc_d = nc.dram_tensor("wfcc", [4, P, V], f16, kind="ExternalInput").ap()
    wfcd_d = nc.dram_tensor("wfcd", [4, P, V], f16, kind="ExternalInput").ap()
    blc_d = nc.dram_tensor("blc", [P, G3], f32, kind="ExternalInput").ap()
    b0hn_d = nc.dram_tensor("b0hn", [P, H], f32, kind="ExternalInput").ap()
    b1rz_d = nc.dram_tensor("b1rz", [P, 2 * H], f32, kind="ExternalInput").ap()
    b1in_d = nc.dram_tensor("b1in", [P, H], f32, kind="ExternalInput").ap()
    b1hn_d = nc.dram_tensor("b1hn", [P, H], f32, kind="ExternalInput").ap()
    bfc_d = nc.dram_tensor("bfc", [P, V], f32, kind="ExternalInput").ap()
    out_d = nc.dram_tensor("out", [P, T, V], f16, kind="ExternalOutput").ap()

    from contextlib import ExitStack
    with tile.TileContext(nc) as tc, ExitStack() as ctx:
        wt = ctx.enter_context(tc.tile_pool(name="wt", bufs=1))
        st = ctx.enter_context(tc.tile_pool(name="st", bufs=1))
        wk = ctx.enter_context(tc.tile_pool(name="wk", bufs=2))
        # PSUM (8 banks): rz 2x[P,1024]f32 double-buffered (4 banks),
        # ihn [P,1024]f32 (2), fc [P,256]f32 (1), f16 transpose scratch (1).
        ps = ctx.enter_context(tc.tile_pool(name="ps", bufs=2, space="PSUM"))
        ps1 = ctx.enter_context(tc.tile_pool(name="ps1", bufs=1, space="PSUM"))

        # ---- persistent weights/biases in SBUF ----
        whh0c = wt.tile([P, 4, G3], f16, tag="whh0c")
        whh0d = wt.tile([P, 4, G3], f16, tag="whh0d")
        wih1c = wt.tile([P, 4, G3], f16, tag="wih1c")
        wih1d = wt.tile([P, 4, G3], f16, tag="wih1d")
        whh1c = wt.tile([P, 4, G3], f16, tag="whh1c")
        whh1d = wt.tile([P, 4, G3], f16, tag="whh1d")
        wembh = wt.tile([P, 2, G3], f16, tag="wembh")
        wembl = wt.tile([P, 2, G3], f16, tag="wembl")
        wlatT = wt.tile([P, 2, G3], f32, tag="wlatT")
        wfcc = wt.tile([P, 4, V], f16, tag="wfcc")
        wfcd = wt.tile([P, 4, V], f16, tag="wfcd")
        # DMA order matches first-use: step 0 needs the Lc chain + wih1 +
        # wfc (hh0/hh1/emb are skipped at t=0), step 1 the rest.
        latsb = wt.tile([P, LAT], f32, tag="latsb")
        nc.sync.dma_start(latsb[:], lat_d[:])
        for kc in range(2):
            nc.sync.dma_start(wlatT[:, kc, :], wlatT_d[kc])
        for kc in range(4):
            nc.sync.dma_start(wih1c[:, kc, :], wih1c_d[kc])
            nc.sync.dma_start(wih1d[:, kc, :], wih1d_d[kc])
        for kc in range(4):
            nc.sync.dma_start(wfcc[:, kc, :], wfcc_d[kc])
            nc.sync.dma_start(wfcd[:, kc, :], wfcd_d[kc])
        for kc in range(4):
            nc.sync.dma_start(whh0c[:, kc, :], whh0c_d[kc])
            nc.sync.dma_start(whh0d[:, kc, :], whh0d_d[kc])
            nc.sync.dma_start(whh1c[:, kc, :], whh1c_d[kc])
            nc.sync.dma_start(whh1d[:, kc, :], whh1d_d[kc])
        for kc in range(2):
            nc.sync.dma_start(wembh[:, kc, :], wembh_d[kc])
            nc.sync.dma_start(wembl[:, kc, :], wembl_d[kc])

        blc = wt.tile([P, G3], f32, tag="blc")
        b0hn = wt.tile([P, H], f32, tag="b0hn")
        b1rz = wt.tile([P, 2 * H], f32, tag="b1rz")
        b1in = wt.tile([P, H], f32, tag="b1in")
        b1hn = wt.tile([P, H], f32, tag="b1hn")
        bfc = wt.tile([P, V], f32, tag="bfc")
        nc.sync.dma_start(blc[:], blc_d[:])
        nc.sync.dma_start(b0hn[:], b0hn_d[:])
        nc.sync.dma_start(b1rz[:], b1rz_d[:])
        nc.sync.dma_start(b1in[:], b1in_d[:])
        nc.sync.dma_start(b1hn[:], b1hn_d[:])
        nc.sync.dma_start(bfc[:], bfc_d[:])

        zer = wt.tile([P, H], bf16, tag="zer")
        nc.gpsimd.memset(zer[:], 0.0)
        ident = wt.tile([P, P], f32, tag="ident")
        make_identity(nc, ident[:])
        identb = wt.tile([P, P], bf16, tag="identb")
        make_identity(nc, identb[:])
        identf = wt.tile([P, P], f16, tag="identf")
        make_identity(nc, identf[:])

        # ---- persistent state ----
        h0 = st.tile([P, H], f32, tag="h0")
        h1 = st.tile([P, H], f32, tag="h1")
        h0Ta = st.tile([P, 4, P], f16, tag="h0Ta")
        h0Tas = st.tile([P, 4, P], f16, tag="h0Tas")
        h0Tb = st.tile([P, 4, P], f16, tag="h0Tb")
        h1Ta = st.tile([P, 4, P], f16, tag="h1Ta")
        h1Tas = st.tile([P, 4, P], f16, tag="h1Tas")
        h1Tb = st.tile([P, 4, P], f16, tag="h1Tb")
        ohT = st.tile([P, 2, P], f16, tag="ohT")
        ohT2 = st.tile([P, 2, P], f16, tag="ohT2")  # one-hot * 2^-12
        Lc = st.tile([P, G3], f32, tag="Lc")
        for tl in (h0, h1):
            nc.gpsimd.memset(tl[:], 0.0)

        # ---- setup: Lc = latent @ WlatT + (b_ih0 + b_hh0 (rz-only)) ----
        s1 = ps.tile([P, 1024], f32, tag="rz")
        latT = wt.tile([P, 2, P], f32, tag="latT")
        for kc in range(2):
            nc.tensor.transpose(s1[:, ts(kc, P)], latsb[:, ts(kc, P)], ident[:])
        nc.scalar.copy(latT[:, :, :].rearrange("p a b -> p (a b)"), s1[:, 0:256])

        s2 = ps.tile([P, 1024], f32, tag="rz")
        s3 = ps1.tile([P, 1024], f32, tag="ihn")
        for kc in range(2):
            for j in range(2):
                nc.tensor.matmul(s2[:, ts(j, 512)], latT[:, kc, :],
                                 wlatT[:, kc, ts(j, 512)],
                                 start=(kc == 0), stop=(kc == 1))
            nc.tensor.matmul(s3[:, 0:512], latT[:, kc, :],
                             wlatT[:, kc, 1024:1536],
                             start=(kc == 0), stop=(kc == 1))
        nc.vector.tensor_add(Lc[:, 0:1024], s2[:, 0:1024], blc[:, 0:1024])
        nc.vector.tensor_add(Lc[:, 1024:1536], s3[:, 0:512], blc[:, 1024:1536])

        def zero_mm(dest):
            """Write zeros to a [P, n] psum region via bf16 zero-matmuls."""
            n = dest.shape[-1]
            for ci in range(0, n, 512):
                w = min(512, n - ci)
                nc.tensor.matmul(dest[:, ci:ci + w], identb[:], zer[:, 0:w],
                                 start=True, stop=True)

        def split_h(h, ha, has, hb, trsp, cols, tag):
            """a=f16(h), b=f16(h-a); transpose both into sbuf [P,4,P] f16,
            plus a*2^-12 (pairs with the *2^12-scaled W residual; exponent
            shift, exact; subnormal tail only perturbs the 2^-12-scale term
            at 2^-24-absolute). trsp: [P,1024] f16 psum scratch."""
            a = wk.tile([P, H], f16, tag="spa", name=f"spa{tag}")
            nc.scalar.copy(a[:], h[:])
            b = wk.tile([P, H], f16, tag="spb", name=f"spb{tag}")
            nc.vector.tensor_sub(b[:], h[:], a[:])
            ab, bb = cols
            for kc in range(4):
                nc.tensor.transpose(trsp[:, ab + kc * P:ab + (kc + 1) * P],
                                    a[:, ts(kc, P)], identf[:])
                nc.scalar.copy(ha[:, kc, :], trsp[:, ab + kc * P:ab + (kc + 1) * P])
                nc.scalar.mul(has[:, kc, :], trsp[:, ab + kc * P:ab + (kc + 1) * P],
                              2.0 ** -12)
                nc.tensor.transpose(trsp[:, bb + kc * P:bb + (kc + 1) * P],
                                    b[:, ts(kc, P)], identf[:])
                nc.scalar.copy(hb[:, kc, :], trsp[:, bb + kc * P:bb + (kc + 1) * P])

        def big_mm(grz, gn, ha, has, hb, wc, wd, gn_sl, first_rz, last_rz,
                   first_n, last_n):
            """3-term f16 split matmul: [rz | n] gates of one K=512 product.
            grz: [P,1024] psum slices j=0,1; gn: psum region, gn_sl slice.
            All rz matmuls go first: the n psum buffer may still be owned
            by the previous layer's gate reads (in-order PE would stall the
            rz prefetch behind an early n matmul)."""
            terms = ((ha, wc), (has, wd), (hb, wc))
            nterm = len(terms)
            for kc in range(4):
                for ti, (s, m) in enumerate(terms):
                    fst = first_rz and kc == 0 and ti == 0
                    lst = last_rz and kc == 3 and ti == nterm - 1
                    for j in range(2):
                        nc.tensor.matmul(grz[:, ts(j, 512)], s[:, kc, :],
                                         m[:, kc, ts(j, 512)],
                                         start=fst, stop=lst)
            for kc in range(4):
                for ti, (s, m) in enumerate(terms):
                    nc.tensor.matmul(gn[:, gn_sl], s[:, kc, :],
                                     m[:, kc, 1024:1536],
                                     start=first_n and kc == 0 and ti == 0,
                                     stop=last_n and kc == 3 and ti == nterm - 1)

        def gru_gates(grz, gihn, rzbias, nbias_i, bhn, h, tag):
            """gates + state update for one layer; h updated in place.
            grz psum [P,1024] holds gi_rz + gh_rz (no bias); rzbias [P,1024]
            fp32 sbuf is added on DVE before the sigmoids. gihn psum:
            [gi_n | gh_n]; nbias_i/bhn are fp32 sbuf adds."""
            rt = wk.tile([P, H], f32, tag="rt", name=f"rt{tag}")
            nc.vector.tensor_add(rt[:], grz[:, 0:512], rzbias[:, 0:512])
            rr = wk.tile([P, H], f32, tag="rr", name=f"rr{tag}")
            nc.scalar.activation(rr[:], rt[:], AF.Sigmoid)
            # off-critical-path adds overlap the sigmoid
            hn = wk.tile([P, H], f32, tag="hn", name=f"hn{tag}")
            nc.vector.tensor_add(hn[:], gihn[:, 512:1024], bhn)
            inn = wk.tile([P, H], f32, tag="inn", name=f"inn{tag}")
            nc.vector.tensor_add(inn[:], gihn[:, 0:512], nbias_i)
            zt = wk.tile([P, H], f32, tag="zt", name=f"zt{tag}")
            nc.vector.tensor_add(zt[:], grz[:, 512:1024], rzbias[:, 512:1024])
            zz = wk.tile([P, H], f32, tag="zz", name=f"zz{tag}")
            nc.scalar.activation(zz[:], zt[:], AF.Sigmoid)
            rhn = wk.tile([P, H], f32, tag="rhn", name=f"rhn{tag}")
            nc.vector.tensor_mul(rhn[:], rr[:], hn[:])
            npre = wk.tile([P, H], f32, tag="npre", name=f"npre{tag}")
            nc.vector.tensor_add(npre[:], inn[:], rhn[:])
            nn = wk.tile([P, H], f32, tag="nn", name=f"nn{tag}")
            nc.scalar.activation(nn[:], npre[:], AF.Tanh)
            # h' = n + z*(h - n)
            dd = wk.tile([P, H], f32, tag="dd", name=f"dd{tag}")
            nc.vector.tensor_sub(dd[:], h[:], nn[:])
            zd = wk.tile([P, H], f32, tag="zd", name=f"zd{tag}")
            nc.vector.tensor_mul(zd[:], zz[:], dd[:])
            for kc in range(4):
                sl = slice(kc * P, (kc + 1) * P)
                nc.vector.tensor_add(h[:, sl], nn[:, sl], zd[:, sl])

        # ---- the T decode steps, software-pipelined: step t's hh0/gh1-rz
        # matmuls are emitted before step t-1's argmax tail, so the PE chews
        # on them while DVE finishes t-1. ----
        def argmax_tail(t, trsp, lg):
            """lg+bias -> argmax -> one-hot f16 (first max wins) -> DMA+ohT."""
            lgb = wk.tile([P, V], f32, tag="lgb", name=f"lgb_{t}")
            nc.vector.tensor_add(lgb[:], lg, bfc[:])
            mx = wk.tile([P, 1], f32, tag="mx", name=f"mx_{t}")
            nc.vector.reduce_max(mx[:], lgb[:], axis=mybir.AxisListType.X)
            ohraw = wk.tile([P, V], f32, tag="ohraw", name=f"ohraw_{t}")
            nc.vector.tensor_scalar(ohraw[:], lgb[:], mx[:, 0:1], None,
                                    op0=OP.is_equal)
            cum = wk.tile([P, V], f32, tag="cum", name=f"cum_{t}")
            nc.vector.tensor_tensor_scan(cum[:], ohraw[:], ohraw[:], 0.0,
                                         op0=OP.add, op1=OP.bypass)
            oh = wk.tile([P, V], f16, tag="oh", name=f"oh_{t}")
            nc.vector.scalar_tensor_tensor(oh[:], cum[:], 1.0, ohraw[:],
                                           op0=OP.is_equal, op1=OP.mult)
            nc.sync.dma_start(out_d[:, t, :], oh[:])
            if trsp is not None:
                for v in range(2):
                    nc.tensor.transpose(trsp[:, v * P:(v + 1) * P],
                                        oh[:, ts(v, P)], identf[:])
                nc.scalar.copy(ohT[:, :, :].rearrange("p a b -> p (a b)"),
                               trsp[:, 0:256])
                nc.scalar.mul(ohT2[:, :, :].rearrange("p a b -> p (a b)"),
                              trsp[:, 0:256], 2.0 ** -12)

        prev_lg = None
        for t in range(T):
            # -- prefetchable matmuls for step t (fill PE during t-1 tail) --
            g0rz = ps.tile([P, 1024], f32, tag="rz", name=f"g0rz_{t}")
            g0ihn = ps1.tile([P, 1024], f32, tag="ihn", name=f"g0ihn_{t}")
            if t > 0:
                big_mm(g0rz, g0ihn, h0Ta, h0Tas, h0Tb, whh0c, whh0d,
                       slice(512, 1024), first_rz=True, last_rz=False,
                       first_n=True, last_n=True)
            else:
                zero_mm(g0rz)
                zero_mm(g0ihn[:, 512:1024])
            g1rz = ps.tile([P, 1024], f32, tag="rz", name=f"g1rz_{t}")
            if t > 0:
                # gh1 rz terms (h1T from t-1); gi1 rz terms close the group
                terms = ((h1Ta, whh1c), (h1Tas, whh1d), (h1Tb, whh1c))
                for kc in range(4):
                    for ti, (s, m) in enumerate(terms):
                        for j in range(2):
                            nc.tensor.matmul(g1rz[:, ts(j, 512)], s[:, kc, :],
                                             m[:, kc, ts(j, 512)],
                                             start=(kc == 0 and ti == 0),
                                             stop=False)
            # at t=0 gh1 is skipped; gi1 opens the g1rz group instead

            # -- step t-1 tail: argmax -> one-hot -> ohT (DVE/ACT work) --
            if t > 0:
                trsp_oh = ps1.tile([P, 1024], f16, tag="trsp",
                                   name=f"trsp_oh_{t}")
                argmax_tail(t - 1, trsp_oh, prev_lg)

            # -- emb finishes layer0 groups (needs ohT from t-1 tail);
            #    EXACT 2-pass fp16; regions complete r -> i_n -> z --
            if t > 0:
                passes = ((ohT, wembh), (ohT2, wembl))
                for j in (0, None, 1):
                    if j is None:
                        for pi, (oh_s, hl) in enumerate(passes):
                            for v in range(2):
                                nc.tensor.matmul(
                                    g0ihn[:, 0:512], oh_s[:, v, :],
                                    hl[:, v, 1024:1536],
                                    start=(pi == 0 and v == 0),
                                    stop=(pi == 1 and v == 1))
                        continue
                    for pi, (oh_s, hl) in enumerate(passes):
                        for v in range(2):
                            nc.tensor.matmul(g0rz[:, ts(j, 512)],
                                             oh_s[:, v, :],
                                             hl[:, v, ts(j, 512)],
                                             start=False,
                                             stop=(pi == 1 and v == 1))
            else:
                zero_mm(g0ihn[:, 0:512])

            # -- layer0 gates -> h0 (in place) --
            gru_gates(g0rz, g0ihn, Lc[:, 0:1024], Lc[:, 1024:1536],
                      b0hn[:], h0, f"0_{t}")

            # -- gh1 h_n: runnable while DVE computes the l0 gates --
            g1ihn = ps1.tile([P, 1024], f32, tag="ihn", name=f"g1ihn_{t}")
            if t > 0:
                terms = ((h1Ta, whh1c), (h1Tas, whh1d), (h1Tb, whh1c))
                for kc in range(4):
                    for ti, (s, m) in enumerate(terms):
                        nc.tensor.matmul(g1ihn[:, 512:1024], s[:, kc, :],
                                         m[:, kc, 1024:1536],
                                         start=(kc == 0 and ti == 0),
                                         stop=(kc == 3 and ti == 2))
            else:
                zero_mm(g1ihn[:, 512:1024])

            # -- h0' split (f16 hi/lo) + transposes -> h0Ta/h0Tb --
            trsp0 = ps1.tile([P, 1024], f16, tag="trsp", name=f"trsp0_{t}")
            split_h(h0, h0Ta, h0Tas, h0Tb, trsp0, (0, 512), f"0_{t}")

            # -- gi1 (= h0' @ Wih1), closes g1rz + fills g1ihn[0:512] --
            big_mm(g1rz, g1ihn, h0Ta, h0Tas, h0Tb, wih1c, wih1d,
                   slice(0, 512), first_rz=(t == 0), last_rz=True,
                   first_n=True, last_n=True)

            # -- layer1 gates -> h1 (in place) --
            gru_gates(g1rz, g1ihn, b1rz[:], b1in[:], b1hn[:], h1, f"1_{t}")

            # -- h1' split + transposes -> h1Ta/h1Tb --
            trsp1 = ps1.tile([P, 1024], f16, tag="trsp", name=f"trsp1_{t}")
            split_h(h1, h1Ta, h1Tas, h1Tb, trsp1, (0, 512), f"1_{t}")

            # -- fc logits (3-term f16) -> fc psum --
            fcp = ps1.tile([P, V], f32, tag="fc", name=f"fc_{t}")
            fterms = ((h1Ta, wfcc), (h1Tas, wfcd), (h1Tb, wfcc))
            for kc in range(4):
                for ti, (s, m) in enumerate(fterms):
                    nc.tensor.matmul(fcp[:], s[:, kc, :], m[:, kc, :],
                                     start=(kc == 0 and ti == 0),
                                     stop=(kc == 3 and ti == 2))
            prev_lg = fcp[:]

        argmax_tail(T - 1, None, prev_lg)

    nc.compile()
    return nc


def prep_host_inputs(latent_vec, w_ih0, w_hh0, b_ih0, b_hh0,
                     w_ih_r, w_hh_r, b_ih_r, b_hh_r, w_fc, b_fc):
    """Pure-layout host prep: transposes/reshapes, f16 pair splits, bias
    merge/replicate. Returns per-core in_maps."""
    f4 = np.float32
    f16 = np.float16

    def rep(v):  # replicate a [N] vector across the 128 partitions
        return np.ascontiguousarray(np.broadcast_to(v.astype(f4), (P, v.shape[0])))

    def split_f16(a):  # c = f16(a), d_s = f16((a-c)*2^12); 3-term operands
        c = a.astype(f16)
        d = ((a - c.astype(f4)) * 4096.0).astype(f16)
        return np.ascontiguousarray(c), np.ascontiguousarray(d)

    wlatT = np.ascontiguousarray(w_ih0[:, :LAT].T.astype(f4)).reshape(2, P, G3)
    wembT = np.ascontiguousarray(w_ih0[:, LAT:].T.astype(f4)).reshape(2, P, G3)
    wembh = wembT.astype(f16)
    wembl = ((wembT - wembh.astype(f4)) * 4096.0).astype(f16)
    wembh, wembl = np.ascontiguousarray(wembh), np.ascontiguousarray(wembl)
    whh0c, whh0d = split_f16(
        np.ascontiguousarray(w_hh0.T.astype(f4)).reshape(4, P, G3))
    wih1c, wih1d = split_f16(
        np.ascontiguousarray(w_ih_r[0].T.astype(f4)).reshape(4, P, G3))
    whh1c, whh1d = split_f16(
        np.ascontiguousarray(w_hh_r[0].T.astype(f4)).reshape(4, P, G3))
    wfcc, wfcd = split_f16(
        np.ascontiguousarray(w_fc.T.astype(f4)).reshape(4, P, V))

    blc_v = b_ih0.astype(f4).copy()
    blc_v[:1024] += b_hh0[:1024].astype(f4)
    common = dict(
        wlatT=wlatT, wembh=wembh, wembl=wembl,
        whh0c=whh0c, whh0d=whh0d, wih1c=wih1c, wih1d=wih1d,
        whh1c=whh1c, whh1d=whh1d, wfcc=wfcc, wfcd=wfcd,
        blc=rep(blc_v), b0hn=rep(b_hh0[1024:]),
        b1rz=rep((b_ih_r[0] + b_hh_r[0])[:1024]),
        b1in=rep(b_ih_r[0][1024:]), b1hn=rep(b_hh_r[0][1024:]),
        bfc=rep(b_fc),
    )
    in_maps = []
    for c in range(N_CORES):
        m = dict(common)
        m["lat"] = np.ascontiguousarray(latent_vec[c * P:(c + 1) * P].astype(f4))
        in_maps.append(m)
    return in_maps


def kernel(**inputs):
    from concourse import bass_utils

    key = ("prog", T_FULL)
    if key not in _CACHE:
        _CACHE[key] = build_program(T_FULL)
    nc = _CACHE[key]

    in_maps = prep_host_inputs(
        np.asarray(inputs["latent_vec"]), np.asarray(inputs["w_ih0"]),
        np.asarray(inputs["w_hh0"]), np.asarray(inputs["b_ih0"]),
        np.asarray(inputs["b_hh0"]), np.asarray(inputs["w_ih_r"]),
        np.asarray(inputs["w_hh_r"]), np.asarray(inputs["b_ih_r"]),
        np.asarray(inputs["b_hh_r"]), np.asarray(inputs["w_fc"]),
        np.asarray(inputs["b_fc"]))

    res = bass_utils.run_bass_kernel_spmd(nc, in_maps, list(range(N_CORES)))
    out = np.concatenate([res.results[c]["out"] for c in range(N_CORES)], axis=0)
    return out.astype(np.float32)


# revision 10
# speedup vs baseline: 1.0968x; 1.0275x over previous
"""GRU free-run greedy decoder on 8 Trainium2 NeuronCores (data parallel).

Problem: 2-layer GRU (H=512) + fc(V=256) greedy decode, T=64 steps,
B=1024 batch, latent LAT=256 concatenated with previous one-hot as input.

Sharding: pure data parallel. Each of the 8 cores handles 128 batch rows
(= exactly the 128 SBUF partitions). GRU + fc weights are replicated.
The whole recurrence runs on-chip; only the final [128, T, V] one-hot
stream is DMA'd out (as fp16, exact for one-hots).

Matmul mapping: out[batch, outdim] = lhsT.T @ rhs with
  lhsT (stationary) = activation^T chunk [K=128, 128 batch]
  rhs  (moving)     = weight^T chunk     [K=128, <=512 outdim]

Precision: h-dependent matmuls (hh0 / ih1 / hh1 / fc) run as 3-term fp16
split products accumulated in fp32 PSUM:
    h @ W ~= a@c + (a*2^-12)@d_s + b@c,
    a=f16(h), b=f16(h-a), c=f16(W), d_s=f16((W-c)*2^12)
The scaled d_s pair keeps the W residual at full fp16 precision (W
captured to ~2^-23); b sits partly in fp16 subnormal range, which the PE
honors exactly (verified by HW probe) and is quantum-2^-24-absolute, so
h is captured to ~2^-24 too. The numpy-emulated trajectory of this
scheme matches the fp64 reference argmax for every token, while all
1/2-term variants flip hundreds of tokens. Cost: 12 fp16 chunk-streams
per 512-K matmul vs fp32's 16 cycle-equivalents, i.e. 25% less PE time
on the dominant matmuls plus cheaper transposes/copies.

One-hot embedding stays an EXACT 2-pass fp16 scheme; Lc / layer-1 / fc
biases are added on the Vector engine (frees all PE bias-seed matmuls).
"""

import sys
import numpy as np

sys.path.insert(0, "/opt/trn_rl_repo")

P = 128          # partitions == per-core batch
H = 512          # hidden
V = 256          # vocab
LAT = 256        # latent dim
G3 = 3 * H       # 1536 gate width
T_FULL = 64
N_CORES = 8

_CACHE = {}


def build_program(T=T_FULL):
    """Build + compile the Bass program. Returns the compiled Bacc object."""
    import concourse.bass as bass
    import concourse.tile as tile
    from concourse import bacc, mybir
    from concourse.masks import make_identity

    f32 = mybir.dt.float32
    f16 = mybir.dt.float16
    bf16 = mybir.dt.bfloat16
    AF = mybir.ActivationFunctionType
    OP = mybir.AluOpType
    ts = bass.ts

    nc = bacc.Bacc(
        "TRN2", target_bir_lowering=False, debug=False,
        enable_asserts=False, num_devices=N_CORES,
    )

    # ---- DRAM I/O ----
    lat_d = nc.dram_tensor("lat", [P, LAT], f32, kind="ExternalInput").ap()
    wlatT_d = nc.dram_tensor("wlatT", [2, P, G3], f32, kind="ExternalInput").ap()
    wembh_d = nc.dram_tensor("wembh", [2, P, G3], f16, kind="ExternalInput").ap()
    wembl_d = nc.dram_tensor("wembl", [2, P, G3], f16, kind="ExternalInput").ap()
    whh0c_d = nc.dram_tensor("whh0c", [4, P, G3], f16, kind="ExternalInput").ap()
    whh0d_d = nc.dram_tensor("whh0d", [4, P, G3], f16, kind="ExternalInput").ap()
    wih1c_d = nc.dram_tensor("wih1c", [4, P, G3], f16, kind="ExternalInput").ap()
    wih1d_d = nc.dram_tensor("wih1d", [4, P, G3], f16, kind="ExternalInput").ap()
    whh1c_d = nc.dram_tensor("whh1c", [4, P, G3], f16, kind="ExternalInput").ap()
    whh1d_d = nc.dram_tensor("whh1d", [4, P, G3], f16, kind="ExternalInput").ap()
    wfcc_d = nc.dram_tensor("wfcc", [4, P, V], f16, kind="ExternalInput").ap()
    wfcd_d = nc.dram_tensor("wfcd", [4, P, V], f16, kind="ExternalInput").ap()
    blc_d = nc.dram_tensor("blc", [P, G3], f32, kind="ExternalInput").ap()
    b0hn_d = nc.dram_tensor("b0hn", [P, H], f32, kind="ExternalInput").ap()
    b1rz_d = nc.dram_tensor("b1rz", [P, 2 * H], f32, kind="ExternalInput").ap()
    b1nb_d = nc.dram_tensor("b1nb", [P, 2 * H], f32, kind="ExternalInput").ap()
    bfc_d = nc.dram_tensor("bfc", [P, V], f32, kind="ExternalInput").ap()
    out_d = nc.dram_tensor("out", [P, T, V], f16, kind="ExternalOutput").ap()

    from contextlib import ExitStack
    with tile.TileContext(nc) as tc, ExitStack() as ctx:
        wt = ctx.enter_context(tc.tile_pool(name="wt", bufs=1))
        st = ctx.enter_context(tc.tile_pool(name="st", bufs=1))
        wk = ctx.enter_context(tc.tile_pool(name="wk", bufs=2))
        # PSUM (8 banks): rz 2x[P,1024]f32 double-buffered (4 banks),
        # ihn [P,1024]f32 (2), fc [P,256]f32 (1), f16 transpose scratch (1).
        ps = ctx.enter_context(tc.tile_pool(name="ps", bufs=2, space="PSUM"))
        ps1 = ctx.enter_context(tc.tile_pool(name="ps1", bufs=1, space="PSUM"))

        # ---- persistent weights/biases in SBUF ----
        whh0c = wt.tile([P, 4, G3], f16, tag="whh0c")
        whh0d = wt.tile([P, 4, G3], f16, tag="whh0d")
        wih1c = wt.tile([P, 4, G3], f16, tag="wih1c")
        wih1d = wt.tile([P, 4, G3], f16, tag="wih1d")
        whh1c = wt.tile([P, 4, G3], f16, tag="whh1c")
        whh1d = wt.tile([P, 4, G3], f16, tag="whh1d")
        wembh = wt.tile([P, 2, G3], f16, tag="wembh")
        wembl = wt.tile([P, 2, G3], f16, tag="wembl")
        wlatT = wt.tile([P, 2, G3], f32, tag="wlatT")
        wfcc = wt.tile([P, 4, V], f16, tag="wfcc")
        wfcd = wt.tile([P, 4, V], f16, tag="wfcd")
        # DMA order matches first-use: step 0 needs the Lc chain + wih1 +
        # wfc (hh0/hh1/emb are skipped at t=0), step 1 the rest.
        latsb = wt.tile([P, LAT], f32, tag="latsb")
        nc.sync.dma_start(latsb[:], lat_d[:])
        for kc in range(2):
            nc.sync.dma_start(wlatT[:, kc, :], wlatT_d[kc])
        for kc in range(4):
            nc.sync.dma_start(wih1c[:, kc, :], wih1c_d[kc])
            nc.sync.dma_start(wih1d[:, kc, :], wih1d_d[kc])
        for kc in range(4):
            nc.sync.dma_start(wfcc[:, kc, :], wfcc_d[kc])
            nc.sync.dma_start(wfcd[:, kc, :], wfcd_d[kc])
        for kc in range(4):
            nc.sync.dma_start(whh0c[:, kc, :], whh0c_d[kc])
            nc.sync.dma_start(whh0d[:, kc, :], whh0d_d[kc])
            nc.sync.dma_start(whh1c[:, kc, :], whh1c_d[kc])
            nc.sync.dma_start(whh1d[:, kc, :], whh1d_d[kc])
        for kc in range(2):
            nc.sync.dma_start(wembh[:, kc, :], wembh_d[kc])
            nc.sync.dma_start(wembl[:, kc, :], wembl_d[kc])

        blc = wt.tile([P, G3], f32, tag="blc")
        b0hn = wt.tile([P, H], f32, tag="b0hn")
        b1rz = wt.tile([P, 2 * H], f32, tag="b1rz")
        b1nb = wt.tile([P, 2 * H], f32, tag="b1nb")
        bfc = wt.tile([P, V], f32, tag="bfc")
        nc.sync.dma_start(blc[:], blc_d[:])
        nc.sync.dma_start(b0hn[:], b0hn_d[:])
        nc.sync.dma_start(b1rz[:], b1rz_d[:])
        nc.sync.dma_start(b1nb[:], b1nb_d[:])
        nc.sync.dma_start(bfc[:], bfc_d[:])

        zer = wt.tile([P, H], bf16, tag="zer")
        nc.gpsimd.memset(zer[:], 0.0)
        ident = wt.tile([P, P], f32, tag="ident")
        make_identity(nc, ident[:])
        identb = wt.tile([P, P], bf16, tag="identb")
        make_identity(nc, identb[:])
        identf = wt.tile([P, P], f16, tag="identf")
        make_identity(nc, identf[:])

        # ---- persistent state ----
        h0 = st.tile([P, H], f32, tag="h0")
        h1 = st.tile([P, H], f32, tag="h1")
        h0Ta = st.tile([P, 4, P], f16, tag="h0Ta")
        h0Tas = st.tile([P, 4, P], f16, tag="h0Tas")
        h0Tb = st.tile([P, 4, P], f16, tag="h0Tb")
        h1Ta = st.tile([P, 4, P], f16, tag="h1Ta")
        h1Tas = st.tile([P, 4, P], f16, tag="h1Tas")
        h1Tb = st.tile([P, 4, P], f16, tag="h1Tb")
        ohT = st.tile([P, 2, P], f16, tag="ohT")
        Lc = st.tile([P, G3], f32, tag="Lc")
        for tl in (h0, h1):
            nc.gpsimd.memset(tl[:], 0.0)

        # ---- setup: Lc = latent @ WlatT + (b_ih0 + b_hh0 (rz-only)) ----
        s1 = ps.tile([P, 1024], f32, tag="rz")
        latT = wt.tile([P, 2, P], f32, tag="latT")
        for kc in range(2):
            nc.tensor.transpose(s1[:, ts(kc, P)], latsb[:, ts(kc, P)], ident[:])
        nc.scalar.copy(latT[:, :, :].rearrange("p a b -> p (a b)"), s1[:, 0:256])

        s2 = ps.tile([P, 1024], f32, tag="rz")
        s3 = ps1.tile([P, 1024], f32, tag="ihn")
        for kc in range(2):
            for j in range(2):
                nc.tensor.matmul(s2[:, ts(j, 512)], latT[:, kc, :],
                                 wlatT[:, kc, ts(j, 512)],
                                 start=(kc == 0), stop=(kc == 1))
            nc.tensor.matmul(s3[:, 0:512], latT[:, kc, :],
                             wlatT[:, kc, 1024:1536],
                             start=(kc == 0), stop=(kc == 1))
        nc.vector.tensor_add(Lc[:, 0:1024], s2[:, 0:1024], blc[:, 0:1024])
        nc.vector.tensor_add(Lc[:, 1024:1536], s3[:, 0:512], blc[:, 1024:1536])
        # combined [i_n-bias | h_n-bias] for the layer-0 single ihn add
        nb0 = wt.tile([P, 2 * H], f32, tag="nb0")
        nc.vector.tensor_copy(nb0[:, 0:512], Lc[:, 1024:1536])
        nc.vector.tensor_copy(nb0[:, 512:1024], b0hn[:])

        def zero_mm(dest):
            """Write zeros to a [P, n] psum region via bf16 zero-matmuls."""
            n = dest.shape[-1]
            for ci in range(0, n, 512):
                w = min(512, n - ci)
                nc.tensor.matmul(dest[:, ci:ci + w], identb[:], zer[:, 0:w],
                                 start=True, stop=True)

        def split_h(h, ha, has, hb, trsp, cols, tag):
            """a=f16(h), b=f16(h-a); transpose both into sbuf [P,4,P] f16,
            plus a*2^-12 (pairs with the *2^12-scaled W residual; exponent
            shift, exact; subnormal tail only perturbs the 2^-12-scale term
            at 2^-24-absolute). trsp: [P,1024] f16 psum scratch."""
            a = wk.tile([P, H], f16, tag="spa", name=f"spa{tag}")
            nc.scalar.copy(a[:], h[:])
            b = wk.tile([P, H], f16, tag="spb", name=f"spb{tag}")
            nc.vector.tensor_sub(b[:], h[:], a[:])
            ab, bb = cols
            for kc in range(4):
                nc.tensor.transpose(trsp[:, ab + kc * P:ab + (kc + 1) * P],
                                    a[:, ts(kc, P)], identf[:])
                nc.tensor.transpose(trsp[:, bb + kc * P:bb + (kc + 1) * P],
                                    b[:, ts(kc, P)], identf[:])
            hav = ha[:, :, :].rearrange("p a b -> p (a b)")
            nc.scalar.copy(hav, trsp[:, ab:ab + 512])
            nc.scalar.mul(has[:, :, :].rearrange("p a b -> p (a b)"),
                          trsp[:, ab:ab + 512], 2.0 ** -12)
            nc.scalar.copy(hb[:, :, :].rearrange("p a b -> p (a b)"),
                           trsp[:, bb:bb + 512])

        def big_mm(grz, gn, ha, has, hb, wc, wd, gn_sl, first_rz, last_rz,
                   first_n, last_n):
            """3-term f16 split matmul: [rz | n] gates of one K=512 product.
            grz: [P,1024] psum slices j=0,1; gn: psum region, gn_sl slice.
            All rz matmuls go first: the n psum buffer may still be owned
            by the previous layer's gate reads (in-order PE would stall the
            rz prefetch behind an early n matmul)."""
            terms = ((ha, wc), (has, wd), (hb, wc))
            nterm = len(terms)
            for kc in range(4):
                for ti, (s, m) in enumerate(terms):
                    fst = first_rz and kc == 0 and ti == 0
                    lst = last_rz and kc == 3 and ti == nterm - 1
                    for j in range(2):
                        nc.tensor.matmul(grz[:, ts(j, 512)], s[:, kc, :],
                                         m[:, kc, ts(j, 512)],
                                         start=fst, stop=lst)
            for kc in range(4):
                for ti, (s, m) in enumerate(terms):
                    nc.tensor.matmul(gn[:, gn_sl], s[:, kc, :],
                                     m[:, kc, 1024:1536],
                                     start=first_n and kc == 0 and ti == 0,
                                     stop=last_n and kc == 3 and ti == nterm - 1)

        def gru_gates(grz, gihn, rzbias, nb, h, tag):
            """gates + state update for one layer; h updated in place.
            grz psum [P,1024] holds gi_rz + gh_rz (no bias); rzbias [P,1024]
            fp32 sbuf is added on DVE before the sigmoids. gihn psum:
            [gi_n | gh_n]; nb = [i_n-bias | h_n-bias] single-add tile."""
            rzt = wk.tile([P, 2 * H], f32, tag="rzt", name=f"rzt{tag}")
            nc.vector.tensor_add(rzt[:], grz[:, 0:1024], rzbias)
            rr = wk.tile([P, H], f32, tag="rr", name=f"rr{tag}")
            nc.scalar.activation(rr[:], rzt[:, 0:512], AF.Sigmoid)
            iht = wk.tile([P, 2 * H], f32, tag="iht", name=f"iht{tag}")
            nc.vector.tensor_add(iht[:], gihn[:, 0:1024], nb)
            zz = wk.tile([P, H], f32, tag="zz", name=f"zz{tag}")
            nc.scalar.activation(zz[:], rzt[:, 512:1024], AF.Sigmoid)
            rhn = wk.tile([P, H], f32, tag="rhn", name=f"rhn{tag}")
            nc.vector.tensor_mul(rhn[:], rr[:], iht[:, 512:1024])
            npre = wk.tile([P, H], f32, tag="npre", name=f"npre{tag}")
            nc.vector.tensor_add(npre[:], iht[:, 0:512], rhn[:])
            nn = wk.tile([P, H], f32, tag="nn", name=f"nn{tag}")
            nc.scalar.activation(nn[:], npre[:], AF.Tanh)
            # h' = n + z*(h - n)
            dd = wk.tile([P, H], f32, tag="dd", name=f"dd{tag}")
            nc.vector.tensor_sub(dd[:], h[:], nn[:])
            zd = wk.tile([P, H], f32, tag="zd", name=f"zd{tag}")
            nc.vector.tensor_mul(zd[:], zz[:], dd[:])
            for kc in range(4):
                sl = slice(kc * P, (kc + 1) * P)
                nc.vector.tensor_add(h[:, sl], nn[:, sl], zd[:, sl])

        # ---- the T decode steps, software-pipelined: step t's hh0/gh1-rz
        # matmuls are emitted before step t-1's argmax tail, so the PE chews
        # on them while DVE finishes t-1. ----
        def argmax_tail(t, trsp, lg):
            """lg+bias -> argmax -> one-hot f16 -> DMA + ohT.
            Fused: one DVE op adds the bias and reduces the row max; a
            second emits the one-hot. Exact fp32 logit ties never occur on
            this trajectory (checked: min top1-top2 gap is 7e-7 >> the
            ~3e-8 kernel error), so is_equal marks exactly one element."""
            lgb = wk.tile([P, V], f32, tag="lgb", name=f"lgb_{t}")
            nc.vector.tensor_add(lgb[:], lg, bfc[:])
            mx = wk.tile([P, 1], f32, tag="mx", name=f"mx_{t}")
            nc.vector.reduce_max(mx[:], lgb[:], axis=mybir.AxisListType.X)
            oh = wk.tile([P, V], f16, tag="oh", name=f"oh_{t}")
            nc.vector.tensor_scalar(oh[:], lgb[:], mx[:, 0:1], None,
                                    op0=OP.is_equal)
            nc.sync.dma_start(out_d[:, t, :], oh[:])
            if trsp is not None:
                for v in range(2):
                    nc.tensor.transpose(trsp[:, v * P:(v + 1) * P],
                                        oh[:, ts(v, P)], identf[:])
                nc.scalar.copy(ohT[:, :, :].rearrange("p a b -> p (a b)"),
                               trsp[:, 0:256])

        prev_lg = None
        for t in range(T):
            # -- prefetchable matmuls for step t (fill PE during t-1 tail) --
            g0rz = ps.tile([P, 1024], f32, tag="rz", name=f"g0rz_{t}")
            g0ihn = ps1.tile([P, 1024], f32, tag="ihn", name=f"g0ihn_{t}")
            if t > 0:
                big_mm(g0rz, g0ihn, h0Ta, h0Tas, h0Tb, whh0c, whh0d,
                       slice(512, 1024), first_rz=True, last_rz=False,
                       first_n=True, last_n=True)
            else:
                zero_mm(g0rz)
                zero_mm(g0ihn[:, 512:1024])
            g1rz = ps.tile([P, 1024], f32, tag="rz", name=f"g1rz_{t}")
            if t > 0:
                # gh1 rz terms (h1T from t-1); gi1 rz terms close the group
                terms = ((h1Ta, whh1c), (h1Tas, whh1d), (h1Tb, whh1c))
                for kc in range(4):
                    for ti, (s, m) in enumerate(terms):
                        for j in range(2):
                            nc.tensor.matmul(g1rz[:, ts(j, 512)], s[:, kc, :],
                                             m[:, kc, ts(j, 512)],
                                             start=(kc == 0 and ti == 0),
                                             stop=False)
            # at t=0 gh1 is skipped; gi1 opens the g1rz group instead

            # -- step t-1 tail: argmax -> one-hot -> ohT (DVE/ACT work) --
            if t > 0:
                trsp_oh = ps1.tile([P, 1024], f16, tag="trsp",
                                   name=f"trsp_oh_{t}")
                argmax_tail(t - 1, trsp_oh, prev_lg)

            # -- emb finishes layer0 groups (needs ohT from t-1 tail);
            #    EXACT 2-pass fp16; regions complete r -> i_n -> z --
            if t > 0:
                passes = ((ohT, wembh), (ohT, wembl))
                for j in (0, None, 1):
                    if j is None:
                        for pi, (oh_s, hl) in enumerate(passes):
                            for v in range(2):
                                nc.tensor.matmul(
                                    g0ihn[:, 0:512], oh_s[:, v, :],
                                    hl[:, v, 1024:1536],
                                    start=(pi == 0 and v == 0),
                                    stop=(pi == 1 and v == 1))
                        continue
                    for pi, (oh_s, hl) in enumerate(passes):
                        for v in range(2):
                            nc.tensor.matmul(g0rz[:, ts(j, 512)],
                                             oh_s[:, v, :],
                                             hl[:, v, ts(j, 512)],
                                             start=False,
                                             stop=(pi == 1 and v == 1))
            else:
                zero_mm(g0ihn[:, 0:512])

            # -- layer0 gates -> h0 (in place) --
            gru_gates(g0rz, g0ihn, Lc[:, 0:1024], nb0[:], h0, f"0_{t}")

            # -- gh1 h_n: runnable while DVE computes the l0 gates --
            g1ihn = ps1.tile([P, 1024], f32, tag="ihn", name=f"g1ihn_{t}")
            if t > 0:
                terms = ((h1Ta, whh1c), (h1Tas, whh1d), (h1Tb, whh1c))
                for kc in range(4):
                    for ti, (s, m) in enumerate(terms):
                        nc.tensor.matmul(g1ihn[:, 512:1024], s[:, kc, :],
                                         m[:, kc, 1024:1536],
                                         start=(kc == 0 and ti == 0),
                                         stop=(kc == 3 and ti == 2))
            else:
                zero_mm(g1ihn[:, 512:1024])

            # -- h0' split (f16 hi/lo) + transposes -> h0Ta/h0Tb --
            trsp0 = ps1.tile([P, 1024], f16, tag="trsp", name=f"trsp0_{t}")
            split_h(h0, h0Ta, h0Tas, h0Tb, trsp0, (0, 512), f"0_{t}")

            # -- gi1 (= h0' @ Wih1), closes g1rz + fills g1ihn[0:512] --
            big_mm(g1rz, g1ihn, h0Ta, h0Tas, h0Tb, wih1c, wih1d,
                   slice(0, 512), first_rz=(t == 0), last_rz=True,
                   first_n=True, last_n=True)

            # -- layer1 gates -> h1 (in place) --
            gru_gates(g1rz, g1ihn, b1rz[:], b1nb[:], h1, f"1_{t}")

            # -- h1' split + transposes -> h1Ta/h1Tb --
            trsp1 = ps1.tile([P, 1024], f16, tag="trsp", name=f"trsp1_{t}")
            split_h(h1, h1Ta, h1Tas, h1Tb, trsp1, (0, 512), f"1_{t}")

            # -- fc logits (3-term f16) -> fc psum --
            fcp = ps1.tile([P, V], f32, tag="fc", name=f"fc_{t}")
            fterms = ((h1Ta, wfcc), (h1Tas, wfcd), (h1Tb, wfcc))
            for kc in range(4):
                for ti, (s, m) in enumerate(fterms):
                    nc.tensor.matmul(fcp[:], s[:, kc, :], m[:, kc, :],
                                     start=(kc == 0 and ti == 0),
                                     stop=(kc == 3 and ti == 2))
            prev_lg = fcp[:]

        argmax_tail(T - 1, None, prev_lg)

    nc.compile()
    return nc


def prep_host_inputs(latent_vec, w_ih0, w_hh0, b_ih0, b_hh0,
                     w_ih_r, w_hh_r, b_ih_r, b_hh_r, w_fc, b_fc):
    """Pure-layout host prep: transposes/reshapes, f16 pair splits, bias
    merge/replicate. Returns per-core in_maps."""
    f4 = np.float32
    f16 = np.float16

    def rep(v):  # replicate a [N] vector across the 128 partitions
        return np.ascontiguousarray(np.broadcast_to(v.astype(f4), (P, v.shape[0])))

    def split_f16(a):  # c = f16(a), d_s = f16((a-c)*2^12); 3-term operands
        c = a.astype(f16)
        d = ((a - c.astype(f4)) * 4096.0).astype(f16)
        return np.ascontiguousarray(c), np.ascontiguousarray(d)

    wlatT = np.ascontiguousarray(w_ih0[:, :LAT].T.astype(f4)).reshape(2, P, G3)
    wembT = np.ascontiguousarray(w_ih0[:, LAT:].T.astype(f4)).reshape(2, P, G3)
    wembh = wembT.astype(f16)
    # residual stored unscaled: fp16 subnormals are honored by the PE and
    # carry 2^-24-absolute quanta, exact enough for the one-hot selection
    wembl = (wembT - wembh.astype(f4)).astype(f16)
    wembh, wembl = np.ascontiguousarray(wembh), np.ascontiguousarray(wembl)
    whh0c, whh0d = split_f16(
        np.ascontiguousarray(w_hh0.T.astype(f4)).reshape(4, P, G3))
    wih1c, wih1d = split_f16(
        np.ascontiguousarray(w_ih_r[0].T.astype(f4)).reshape(4, P, G3))
    whh1c, whh1d = split_f16(
        np.ascontiguousarray(w_hh_r[0].T.astype(f4)).reshape(4, P, G3))
    wfcc, wfcd = split_f16(
        np.ascontiguousarray(w_fc.T.astype(f4)).reshape(4, P, V))

    blc_v = b_ih0.astype(f4).copy()
    blc_v[:1024] += b_hh0[:1024].astype(f4)
    common = dict(
        wlatT=wlatT, wembh=wembh, wembl=wembl,
        whh0c=whh0c, whh0d=whh0d, wih1c=wih1c, wih1d=wih1d,
        whh1c=whh1c, whh1d=whh1d, wfcc=wfcc, wfcd=wfcd,
        blc=rep(blc_v), b0hn=rep(b_hh0[1024:]),
        b1rz=rep((b_ih_r[0] + b_hh_r[0])[:1024]),
        b1nb=rep(np.concatenate([b_ih_r[0][1024:], b_hh_r[0][1024:]])),
        bfc=rep(b_fc),
    )
    in_maps = []
    for c in range(N_CORES):
        m = dict(common)
        m["lat"] = np.ascontiguousarray(latent_vec[c * P:(c + 1) * P].astype(f4))
        in_maps.append(m)
    return in_maps


def kernel(**inputs):
    from concourse import bass_utils

    key = ("prog", T_FULL)
    if key not in _CACHE:
        _CACHE[key] = build_program(T_FULL)
    nc = _CACHE[key]

    in_maps = prep_host_inputs(
        np.asarray(inputs["latent_vec"]), np.asarray(inputs["w_ih0"]),
        np.asarray(inputs["w_hh0"]), np.asarray(inputs["b_ih0"]),
        np.asarray(inputs["b_hh0"]), np.asarray(inputs["w_ih_r"]),
        np.asarray(inputs["w_hh_r"]), np.asarray(inputs["b_ih_r"]),
        np.asarray(inputs["b_hh_r"]), np.asarray(inputs["w_fc"]),
        np.asarray(inputs["b_fc"]))

    res = bass_utils.run_bass_kernel_spmd(nc, in_maps, list(range(N_CORES)))
    out = np.concatenate([res.results[c]["out"] for c in range(N_CORES)], axis=0)
    return out.astype(np.float32)
